# revision 1
# baseline (speedup 1.0000x reference)
"""Trainium2 Bass kernel for nn_AttentionBlock (B=8, H=W=32, C=512, 8 heads).

Strategy: data-parallel over batch -- each of the 8 NeuronCores processes one
batch element end-to-end (no collectives).  Per core:

  x [T=1024, C=512] -> qkv -> per-head attention (T x T softmax) -> out proj.

All matmuls run as float32r (TF32) on the PE at full rate with fp32 PSUM
accumulation.  Softmax is computed in the S^T ([s, t]) layout so the
softmax reduction axis lands on the PSUM partition axis, where the
denominators come for free from a ones-column appended to V during the PV
matmul.  No max-subtraction is needed: logits are ~N(0, 1) by construction
(exp is evaluated with the 1/8 scale folded into the ScalarE activation).
"""

import math
import os
from contextlib import ExitStack

import numpy as np

import concourse.bass as bass
import concourse.mybir as mybir
import concourse.tile as tile
from concourse import bacc

T = 1024          # tokens per batch element (32*32)
C = 512           # channels
HEADS = 8
HC = C // HEADS   # 64
P = 128           # partitions
NT = T // P       # 8 t-tiles
NCT = C // P      # 4 c-tiles
CHUNK = 512       # moving-operand chunk (fp32 max, = one PSUM bank)
NCH = T // CHUNK  # 2 chunks
F32 = mybir.dt.float32
F32R = mybir.dt.float32r
EXP_SCALE = 1.0 / math.sqrt(HC)  # (1/sqrt(sqrt(hc)))^2 applied to q·k
ACT_GROUP = 2     # S^T chunks per ScalarE exp call (2 PSUM banks)


def tf32_round(a: np.ndarray) -> np.ndarray:
    """Round fp32 -> tf32 (10-bit mantissa) with round-to-nearest-even."""
    bits = a.astype(np.float32).view(np.uint32)
    round_bit = np.uint32(1 << 12)
    lsb = (bits >> np.uint32(13)) & np.uint32(1)
    bits = bits + (round_bit - np.uint32(1)) + lsb
    bits &= np.uint32(0xFFFFE000)
    return bits.view(np.float32)


def build_program(debug_dumps: bool = False):
    nc = bacc.Bacc("TRN2", num_devices=8, debug=False)

    x_d = nc.dram_tensor("x", [T, C], F32, kind="ExternalInput")
    wqkv_d = nc.dram_tensor("qkv_w", [C, 3 * C], F32R, kind="ExternalInput")
    wout_d = nc.dram_tensor("out_w", [C, C], F32R, kind="ExternalInput")
    qkb_d = nc.dram_tensor("qk_b", [2 * C], F32, kind="ExternalInput")
    ob_d = nc.dram_tensor("out_b", [C], F32, kind="ExternalInput")
    out_d = nc.dram_tensor("out", [T, C], F32, kind="ExternalOutput")
    dbg = {}
    if debug_dumps:
        for nm, shp in [
            ("dbg_xT", [P, T]), ("dbg_qT", [P, T]), ("dbg_kT", [P, T]),
            ("dbg_v", [P, HEADS * (HC + 1)]), ("dbg_ex", [P, 3 * CHUNK]),
            ("dbg_pv", [HC + 1, CHUNK]), ("dbg_an", [P, T]),
            ("dbg_recip", [1, T]), ("dbg_bcast", [HC, T]),
        ]:
            dbg[nm] = nc.dram_tensor(nm, shp, F32, kind="ExternalOutput")

    with tile.TileContext(nc) as tc, ExitStack() as ctx:
        from concourse.masks import make_identity

        # ---------------- SBUF pools (whole-kernel lifetime) ----------------
        const = ctx.enter_context(tc.tile_pool(name="const", bufs=1))
        persist = ctx.enter_context(tc.tile_pool(name="persist", bufs=1))
        workp = ctx.enter_context(tc.tile_pool(name="workp", bufs=1))
        xload_cm = tc.tile_pool(name="xload", bufs=1)
        xload = xload_cm.__enter__()

        # x tiles first: they gate the transpose pipeline
        xts = []
        for i in range(NT):
            xt_in = xload.tile([P, C], F32, tag=f"x_in{i}", name=f"x_in{i}")
            nc.sync.dma_start(xt_in[:], x_d.ap()[i * P:(i + 1) * P, :])
            xts.append(xt_in)

        # weights straight into fp32r tiles (host pre-rounds the data);
        # v columns land first so the v matmuls can start early
        wq = []  # [c-tile][128, 1536]
        for m in range(NCT):
            t_ = persist.tile([P, 3 * C], F32R, tag=f"wq{m}", name=f"wq{m}")
            nc.gpsimd.dma_start(t_[:, 2 * C:3 * C],
                                wqkv_d.ap()[m * P:(m + 1) * P, 2 * C:3 * C])
            wq.append(t_)
        for m in range(NCT):
            nc.gpsimd.dma_start(wq[m][:, 0:2 * C],
                                wqkv_d.ap()[m * P:(m + 1) * P, 0:2 * C])
        identity = const.tile([P, P], F32, tag="ident", name="ident")
        make_identity(nc, identity[:])

        ones8 = const.tile([P, HEADS, 1], F32, tag="ones8", name="ones8")
        nc.gpsimd.memset(ones8[:], 1.0)

        # bias tiles (single gather DMA each); column m = bias[128m:128m+128]
        qkb_all = const.tile([P, 2 * C // P], F32, tag="qkball", name="qkb_all")
        nc.gpsimd.dma_start(
            qkb_all[:], qkb_d.ap().rearrange("(m p) -> p m", p=P)
        )
        qkb_t = [qkb_all[:, m:m + 1] for m in range(2 * C // P)]

        xT = [xload.tile([P, T], F32R, tag=f"xT{m}", name=f"xT{m}") for m in range(NCT)]
        VAW = HEADS * (HC + 1) + (P - HC - 1)  # 128-wide lhsT reads stay in-tile
        vaug = [persist.tile([P, VAW], F32R, tag=f"va{i}", name=f"va{i}") for i in range(NT)]
        qkT = [persist.tile([P, T], F32R, tag=f"qk{m}", name=f"qk{m}") for m in range(C // P)]
        # per-head zero-padded k^T: even heads use rows 0:64 (zeros below),
        # odd heads rows 64:128 (zeros above) so K=128 S^T matmuls pair with
        # the full q^T tile rows directly.
        kTz = [persist.tile([P, T], F32R, tag=f"kz{h}", name=f"kz{h}") for h in range(HEADS)]
        anorm = [persist.tile([P, T], F32R, tag=f"an{m}", name=f"an{m}") for m in range(NCT)]

        # ================= phase 1: x^T, v, q^T/k^T =================
        with tc.tile_pool(name="ps1", bufs=2, space="PSUM") as ps1:
            # x PE transpose; xT[m] = x^T rows [128m,128m+128) [c, t]
            for i in range(NT):
                xt_in = xts[i]
                ps_tr = ps1.tile([P, C], F32, tag="tr", name="ps_tr")
                for m in range(NCT):
                    nc.tensor.transpose(
                        ps_tr[:, m * P:(m + 1) * P],
                        xt_in[:, m * P:(m + 1) * P],
                        identity[:],
                    )
                for m in range(NCT):
                    nc.vector.tensor_copy(
                        xT[m][:, i * P:(i + 1) * P], ps_tr[:, m * P:(m + 1) * P]
                    )

            # v = x @ Wv; vaug[i]: [128(t), 8, 65], [:, h, 64] = 1.0
            for i in range(NT):
                ps_v = ps1.tile([P, C], F32, tag="v", name="ps_v")
                for m in range(NCT):
                    nc.tensor.matmul(
                        ps_v[:],
                        xT[m][:, i * P:(i + 1) * P],
                        wq[m][:, 2 * C:3 * C],
                        start=(m == 0),
                        stop=(m == NCT - 1),
                    )
                va3 = vaug[i][:, 0:HEADS * (HC + 1)].rearrange(
                    "p (h d) -> p h d", d=HC + 1)
                nc.vector.tensor_copy(
                    va3[:, :, 0:HC],
                    ps_v[:].rearrange("p (h d) -> p h d", h=HEADS),
                )
                nc.vector.tensor_copy(va3[:, :, HC:HC + 1], ones8[:])
                nc.vector.tensor_scalar_mul(
                    vaug[i][:, HEADS * (HC + 1):VAW],
                    ps_v[:, 0:VAW - HEADS * (HC + 1)], 0.0)

            # zero-fill the padding halves of kTz
            for h in range(HEADS):
                zlo = 0 if h % 2 == 1 else HC
                nc.vector.tensor_scalar_mul(
                    kTz[h][zlo:zlo + HC, :], wq[0][0:HC, 0:T], 0.0)
            # q^T/k^T: interleave q/k tile order so head-pair p's attention
            # can start as soon as qkT[p] and kTz[2p..2p+1] exist.
            for m in [0, 4, 1, 5, 2, 6, 3, 7]:
                for j in range(NCH):
                    ps_qk = ps1.tile([P, CHUNK], F32, tag="qk", name="ps_qk")
                    for cc in range(NCT):
                        nc.tensor.matmul(
                            ps_qk[:],
                            wq[cc][:, m * P:(m + 1) * P],
                            xT[cc][:, j * CHUNK:(j + 1) * CHUNK],
                            start=(cc == 0),
                            stop=(cc == NCT - 1),
                        )
                    js = slice(j * CHUNK, (j + 1) * CHUNK)
                    if m < NCT:
                        nc.vector.tensor_scalar_add(
                            qkT[m][:, js], ps_qk[:], qkb_t[m][:]
                        )
                    else:
                        hh = 2 * (m - NCT)
                        nc.vector.tensor_scalar_add(
                            kTz[hh][0:HC, js], ps_qk[0:HC, :],
                            qkb_t[m][0:HC],
                        )
                        nc.vector.tensor_scalar_add(
                            kTz[hh + 1][HC:P, js], ps_qk[HC:P, :],
                            qkb_t[m][HC:P],
                        )

            # out-proj weights + bias: only needed in phase 3; load last
            wo = []  # [c-tile][128, 512]
            for m in range(NCT):
                t_ = persist.tile([P, C], F32R, tag=f"wo{m}", name=f"wo{m}")
                nc.sync.dma_start(t_[:], wout_d.ap()[m * P:(m + 1) * P, :])
                wo.append(t_)
            ob_all = const.tile([P, NCT], F32, tag="oball", name="ob_all")
            nc.sync.dma_start(ob_all[:], ob_d.ap().rearrange("(m p) -> p m", p=P))
            ob_t = [ob_all[:, m:m + 1] for m in range(NCT)]

        dbgp = ctx.enter_context(tc.tile_pool(name="dbgp", bufs=1)) if debug_dumps else None
        if debug_dumps:
            cp = dbgp.tile([P, T], F32, tag="dbg", name="dbgcp")
            nc.vector.tensor_copy(cp[:], xT[0][:].bitcast(F32))
            nc.sync.dma_start(dbg["dbg_xT"].ap(), cp[:])
        xload_cm.__exit__(None, None, None)
        if debug_dumps:
            cp2 = dbgp.tile([P, T], F32, tag="dbg", name="dbgcp2")
            nc.vector.tensor_copy(cp2[:], qkT[0][:].bitcast(F32))
            nc.sync.dma_start(dbg["dbg_qT"].ap(), cp2[:])
            cp3 = dbgp.tile([P, T], F32, tag="dbg", name="dbgcp3")
            nc.vector.tensor_copy(cp3[:], kTz[0][:].bitcast(F32))
            nc.sync.dma_start(dbg["dbg_kT"].ap(), cp3[:])
            cp4 = dbgp.tile([P, HEADS * (HC + 1)], F32, tag="dbg", name="dbgcp4")
            nc.vector.tensor_copy(
                cp4[:], vaug[0][:, 0:HEADS * (HC + 1)].bitcast(F32)
            )
            nc.sync.dma_start(dbg["dbg_v"].ap(), cp4[:])

        # ================= phase 2: attention =================
        # Per head: stream the 8 S^T s-tiles through PSUM -> exp into a
        # full-head expS buffer (ACT-bound stretch).  The PREVIOUS head's 16
        # PV matmuls are interleaved two-per-slot between the S^T fills: they
        # are wait-free (their exp inputs completed last head), so they fill
        # the PE gaps and keep the HAM clock warm.
        def emit_pv_slice(ph, ext, ppv, chunks):
            for c in chunks:
                ssi, j = c // NCH, c % NCH
                nc.tensor.matmul(
                    ppv[j][:],
                    vaug[ssi][:, ph * (HC + 1): ph * (HC + 1) + P],
                    ext[:, c * CHUNK:(c + 1) * CHUNK],
                    start=(ssi == NT - 1),
                    stop=(ssi == 0),
                )

        def emit_normalize(ph, ppv):
            aoff = (ph % 2) * HC
            am = ph // 2
            dtmp = workp.tile([1, T], F32, tag="dtmp", name="dtmp")
            for j in range(NCH):
                nc.vector.tensor_copy(
                    dtmp[:, j * CHUNK:(j + 1) * CHUNK],
                    ppv[j][HC:HC + 1, :],
                )
            recip = workp.tile([1, T], F32, tag="recip", name="recip")
            nc.vector.reciprocal_approx_fast(recip[:], dtmp[:])
            bcast = workp.tile([HC, T], F32, tag="bcast", name="bcast")
            nc.gpsimd.partition_broadcast(bcast[:], recip[:], channels=HC)
            if debug_dumps and ph == 0:
                nc.sync.dma_start(dbg["dbg_recip"].ap(), recip[:])
                nc.sync.dma_start(dbg["dbg_bcast"].ap(), bcast[:])
            for j in range(NCH):
                nc.vector.tensor_tensor(
                    anorm[am][aoff:aoff + HC, j * CHUNK:(j + 1) * CHUNK],
                    ppv[j][0:HC, :],
                    bcast[:, j * CHUNK:(j + 1) * CHUNK],
                    op=mybir.AluOpType.mult,
                )

        # chunk stream per head: 16 chunks c=(si, j), ScalarE exp groups of 3
        CGRP = [list(range(g, min(g + 2, 2 * NT))) for g in range(0, 2 * NT, 2)]
        # prev-head PV chunks interleaved per group slot, reversed order
        PV_SLICES = []
        rc = list(reversed(range(2 * NT)))
        kk = 0
        for g in range(len(CGRP)):
            take = 2 if g < len(CGRP) - 1 else len(rc) - kk
            PV_SLICES.append(rc[kk:kk + take])
            kk += take

        with (
            tc.tile_pool(name="expsp", bufs=2) as expsp,
            tc.tile_pool(name="ps_st", bufs=2, space="PSUM") as ps_st,
            tc.tile_pool(name="ps_pv", bufs=2, space="PSUM") as ps_pv,
        ):
            prev = None  # (head, exh, pv tiles) whose PV burst is pending
            for h in range(HEADS):
                qm = h // 2
                pv_ps = [ps_pv.tile([P, CHUNK], F32, tag=f"pv{j}",
                                    name=f"pv{j}") for j in range(NCH)]
                exh = expsp.tile([P, 2 * NT * CHUNK], F32R, tag="exh", name="exh")
                for g, grp in enumerate(CGRP):
                    gw = len(grp)
                    st_ps = ps_st.tile([P, 2 * CHUNK], F32, tag="st", name="st")
                    for b, c in enumerate(grp):
                        ssi, j = c // NCH, c % NCH
                        nc.tensor.matmul(
                            st_ps[:, b * CHUNK:(b + 1) * CHUNK],
                            kTz[h][:, ssi * P:(ssi + 1) * P],
                            qkT[qm][:, j * CHUNK:(j + 1) * CHUNK],
                            start=True,
                            stop=True,
                        )
                    nc.scalar.activation(
                        exh[:, grp[0] * CHUNK:(grp[-1] + 1) * CHUNK],
                        st_ps[:, 0:gw * CHUNK],
                        mybir.ActivationFunctionType.Exp,
                        scale=EXP_SCALE,
                    )
                    if prev is not None:
                        emit_pv_slice(prev[0], prev[1], prev[2], PV_SLICES[g])
                if prev is not None:
                    emit_normalize(prev[0], prev[2])
                if debug_dumps and h == 0:
                    cp5 = dbgp.tile([P, 3 * CHUNK], F32, tag="dbg", name="dbgcp5")
                    nc.vector.tensor_copy(cp5[:, 0:T], exh[:, 0:T].bitcast(F32))
                    nc.vector.tensor_copy(
                        cp5[:, T:3 * CHUNK], exh[:, T:T + CHUNK].bitcast(F32))
                    nc.sync.dma_start(dbg["dbg_ex"].ap(), cp5[:])
                prev = (h, exh, pv_ps)
            # final head: straight burst + normalize
            emit_pv_slice(prev[0], prev[1], prev[2], list(reversed(range(2 * NT))))
            if debug_dumps:
                cp6 = dbgp.tile([HC + 1, CHUNK], F32, tag="dbg", name="dbgcp6")
                nc.vector.tensor_copy(cp6[:], prev[2][0][:])
                nc.sync.dma_start(dbg["dbg_pv"].ap(), cp6[:])
            emit_normalize(prev[0], prev[2])

        if debug_dumps:
            cp7 = dbgp.tile([P, T], F32, tag="dbg", name="dbgcp7")
            nc.vector.tensor_copy(cp7[:], anorm[0][:].bitcast(F32))
            nc.sync.dma_start(dbg["dbg_an"].ap(), cp7[:])

        # ================= phase 3: out projection + transpose =================
        with (
            tc.tile_pool(name="otp", bufs=1) as otp,
            tc.tile_pool(name="ps3", bufs=2, space="PSUM") as ps3,
        ):
            outT = []  # [e-tile][128, 1024] fp32
            for e in range(NCT):
                ps_o = ps3.tile([P, T], F32, tag="o", name="ps_o")
                for j in range(NCH):
                    for cc in range(NCT):
                        nc.tensor.matmul(
                            ps_o[:, j * CHUNK:(j + 1) * CHUNK],
                            wo[cc][:, e * P:(e + 1) * P],
                            anorm[cc][:, j * CHUNK:(j + 1) * CHUNK],
                            start=(cc == 0),
                            stop=(cc == NCT - 1),
                        )
                ot = otp.tile([P, T], F32, tag=f"ot{e}", name=f"ot{e}")
                nc.vector.tensor_scalar_add(ot[:], ps_o[:], ob_t[e][:])
                outT.append(ot)

            for i in range(NT):
                ps_tr = ps3.tile([P, C], F32, tag="tr2", name="ps_tr2")
                for e in range(NCT):
                    nc.tensor.transpose(
                        ps_tr[:, e * P:(e + 1) * P],
                        outT[e][:, i * P:(i + 1) * P],
                        identity[:],
                    )
                of = workp.tile([P, C], F32, tag="of", name="of")
                nc.scalar.copy(of[:], ps_tr[:])
                nc.sync.dma_start(out_d.ap()[i * P:(i + 1) * P, :], of[:])

    nc.compile()
    return nc


_CACHED_NC = None


def _get_nc():
    global _CACHED_NC
    if _CACHED_NC is None:
        _CACHED_NC = build_program()
    return _CACHED_NC


def kernel(x, qkv_w, qkv_b, out_w, out_b):
    """Full inputs in, full output out.  Shards batch across 8 NeuronCores."""
    from concourse.bass_utils import run_bass_kernel_spmd

    x = np.asarray(x)
    B, H, W, Cc = x.shape
    assert (B, H, W, Cc) == (8, 32, 32, C)
    x2 = np.ascontiguousarray(x.reshape(B, T, C).astype(np.float32))
    wq2 = np.asarray(qkv_w).reshape(C, 3 * C).astype(np.float32)
    wo2 = np.asarray(out_w).reshape(C, C).astype(np.float32)
    qkv_b = np.asarray(qkv_b).astype(np.float32)
    out_b = np.asarray(out_b).astype(np.float32)

    # host-side prep: tf32-round the weights (device loads them as float32r),
    # fold the v-bias through the output projection (exact: A_norm += b_v
    # shifts out by b_v @ W_out).
    wq_r = tf32_round(wq2)
    wo_r = tf32_round(wo2)
    b_v = qkv_b[2 * C:3 * C]
    ob_eff = (
        out_b.astype(np.float64) + b_v.astype(np.float64) @ wo_r.astype(np.float64)
    ).astype(np.float32)
    qkb = np.ascontiguousarray(qkv_b[0:2 * C])

    nc = _get_nc()
    in_maps = [
        {
            "x": np.ascontiguousarray(x2[b]),
            "qkv_w": np.ascontiguousarray(wq_r),
            "out_w": np.ascontiguousarray(wo_r),
            "qk_b": qkb,
            "out_b": ob_eff,
        }
        for b in range(B)
    ]
    trace = bool(int(os.environ.get("KERNEL_TRACE", "0")))
    res = run_bass_kernel_spmd(nc, in_maps, core_ids=list(range(B)), trace=trace)
    if trace and res.exec_time_ns is not None:
        print(f"HW exec time: {res.exec_time_ns} ns")
    kernel.last_results = res
    out = np.stack([res.results[b]["out"] for b in range(B)], axis=0)
    return out.reshape(B, H, W, Cc)


kernel.last_results = None



# revision 11
# speedup vs baseline: 1.1547x; 1.1547x over previous
"""Trainium2 Bass kernel for nn_AttentionBlock (B=8, H=W=32, C=512, 8 heads).

Data-parallel over batch: each of 8 NeuronCores does one batch element.

Per core, the kernel is organized around keeping ScalarE (softmax exp, the
serial floor at ~73us of ACT work) 100% busy while TensorE work rides in
its shadow:

  ramp:    x DMA (2 queues) -> PE transposes -> v (all s-tiles) ->
           q^T m=0 / k^T m=4 (head pair 0), exp-table warm-up.
  phase 2: per head h, 8 slots: S^T s-tile fill (2 K=64 matmuls) ->
           ScalarE exp(N=1024) -> same-head PV matmuls lagging one slot.
           Background qkv-projection matmuls for later head pairs are
           woven into the S^T PSUM ring (one m-tile per head).
  phase 3: out-projection directly in [t, e] layout (stationary = A^T
           chunk), DVE bias-add evacuate, DMA out.  No output transpose,
           no ScalarE copies.

Numerics identical to the proven baseline: float32r matmuls with fp32
PSUM accumulation, softmax without max-subtraction (logits ~N(0,1)),
denominators via a ones-column appended to V, host-side tf32 rounding
and v-bias folding into the output bias.
"""

import math
import os
from contextlib import ExitStack

import numpy as np

import concourse.bass as bass
import concourse.mybir as mybir
import concourse.tile as tile
from concourse import bacc

T = 1024          # tokens per batch element (32*32)
C = 512           # channels
HEADS = 8
HC = C // HEADS   # 64
P = 128           # partitions
NT = T // P       # 8 t-tiles
NCT = C // P      # 4 c-tiles
CHUNK = 512       # moving-operand chunk (fp32 max, = one PSUM bank)
NCH = T // CHUNK  # 2 chunks
F32 = mybir.dt.float32
F32R = mybir.dt.float32r
EXP_SCALE = 1.0 / math.sqrt(HC)  # (1/sqrt(sqrt(hc)))^2 applied to q·k
VW = HC + 1       # v channels + ones column


def tf32_round(a: np.ndarray) -> np.ndarray:
    """Round fp32 -> tf32 (10-bit mantissa) with round-to-nearest-even."""
    bits = a.astype(np.float32).view(np.uint32)
    round_bit = np.uint32(1 << 12)
    lsb = (bits >> np.uint32(13)) & np.uint32(1)
    bits = bits + (round_bit - np.uint32(1)) + lsb
    bits &= np.uint32(0xFFFFE000)
    return bits.view(np.float32)


def build_program(debug_dumps: bool = False):
    nc = bacc.Bacc("TRN2", num_devices=8, debug=False)

    x_d = nc.dram_tensor("x", [T, C], F32, kind="ExternalInput")
    wqkv_d = nc.dram_tensor("qkv_w", [C, 3 * C], F32R, kind="ExternalInput")
    wout_d = nc.dram_tensor("out_w", [C, C], F32R, kind="ExternalInput")
    qkb_d = nc.dram_tensor("qk_b", [2 * C], F32, kind="ExternalInput")
    ob_d = nc.dram_tensor("out_b", [C], F32, kind="ExternalInput")
    out_d = nc.dram_tensor("out", [T, C], F32, kind="ExternalOutput")

    with tile.TileContext(nc) as tc, ExitStack() as ctx:
        from concourse.masks import make_identity

        # ---------------- SBUF pools ----------------
        const = ctx.enter_context(tc.tile_pool(name="const", bufs=1))
        persist = ctx.enter_context(tc.tile_pool(name="persist", bufs=1))
        workp = ctx.enter_context(tc.tile_pool(name="workp", bufs=2))
        xin_cm = tc.tile_pool(name="xin", bufs=1)
        xin = xin_cm.__enter__()

        # x tiles: gate the transpose pipeline; split across two DMA queues
        xts = []
        for i in range(NT):
            xt_in = xin.tile([P, C], F32, tag=f"xin{i}", name=f"xin{i}")
            q = nc.sync if i % 2 == 0 else nc.scalar
            q.dma_start(xt_in[:], x_d.ap()[i * P:(i + 1) * P, :])
            xts.append(xt_in)

        # exp table warm-up: tiny activation ASAP so the ~2.7us table load
        # happens during the DMA/transpose ramp, not before the first real exp.
        warm = const.tile([1, 2], F32, tag="warm", name="warm")
        nc.gpsimd.memset(warm[:, 0:1], 0.0)
        nc.scalar.activation(warm[:, 1:2], warm[:, 0:1],
                             mybir.ActivationFunctionType.Exp)

        # weights: head-pair-0 q/k columns first, the rest on a second queue
        wq = []  # [c-tile][128, 1536] (q | k | v)
        for m in range(NCT):
            t_ = persist.tile([P, 3 * C], F32R, tag=f"wq{m}", name=f"wq{m}")
            nc.gpsimd.dma_start(t_[:, 0:640],
                                wqkv_d.ap()[m * P:(m + 1) * P, 0:640])
            wq.append(t_)
        for m in range(NCT):
            nc.gpsimd.dma_start(wq[m][:, 640:3 * C],
                                wqkv_d.ap()[m * P:(m + 1) * P, 640:3 * C])
        wo = []  # [c-tile][128, 512]
        for m in range(NCT):
            t_ = persist.tile([P, C], F32R, tag=f"wo{m}", name=f"wo{m}")
            nc.sync.dma_start(t_[:], wout_d.ap()[m * P:(m + 1) * P, :])
            wo.append(t_)

        identity = const.tile([P, P], F32, tag="ident", name="ident")
        make_identity(nc, identity[:])
        ones8 = const.tile([P, HEADS, 1], F32, tag="ones8", name="ones8")
        nc.gpsimd.memset(ones8[:], 1.0)

        # biases; column m of qkb_all = qkv_b[128m:128m+128]
        qkb_all = const.tile([P, 2 * C // P], F32, tag="qkball", name="qkb_all")
        nc.gpsimd.dma_start(
            qkb_all[:], qkb_d.ap().rearrange("(m p) -> p m", p=P)
        )
        qkb_t = [qkb_all[:, m:m + 1] for m in range(2 * C // P)]
        ob_row = const.tile([1, C], F32, tag="obrow", name="ob_row")
        nc.sync.dma_start(ob_row[:], ob_d.ap().rearrange("(o c) -> o c", o=1))
        obb = const.tile([P, C], F32, tag="obb", name="obb")
        nc.gpsimd.partition_broadcast(obb[:], ob_row[:], channels=P)

        xT = [persist.tile([P, T], F32R, tag=f"xT{m}", name=f"xT{m}")
              for m in range(NCT)]
        VAW = HEADS * VW + (P - VW)  # 128-wide lhsT reads stay in-tile
        vaug = [persist.tile([P, VAW], F32R, tag=f"va{i}", name=f"va{i}")
                for i in range(NT)]
        qkT = [persist.tile([P, T], F32R, tag=f"qk{m}", name=f"qk{m}")
               for m in range(NCT)]
        # per-head zero-padded k^T: even heads use rows 0:64 (zeros below),
        # odd heads rows 64:128, so K=128 S^T matmuls pair with full q^T rows.
        kTz = [persist.tile([P, T], F32R, tag=f"kz{h}", name=f"kz{h}")
               for h in range(HEADS)]
        anorm = [persist.tile([P, T], F32R, tag=f"an{m}", name=f"an{m}")
                 for m in range(NCT)]
        for i in range(NT):
            nc.vector.tensor_scalar_mul(
                vaug[i][:, HEADS * VW:VAW], wq[0][:, 0:VAW - HEADS * VW], 0.0
            )
        for h in range(HEADS):
            zlo = 0 if h % 2 == 1 else HC
            nc.vector.tensor_scalar_mul(
                kTz[h][zlo:zlo + HC, :], wq[0][0:HC, 0:T], 0.0
            )

        # ================= ramp: x^T, v, q^T/k^T pair 0 =================
        ps1_cm = tc.tile_pool(name="ps1", bufs=2, space="PSUM")
        ps1 = ps1_cm.__enter__()

        # x PE transpose; xT[m] = x^T rows [128m,128m+128) [c, t]
        for i in range(NT):
            ps_tr = ps1.tile([P, C], F32, tag="tr", name="ps_tr")
            for m in range(NCT):
                nc.tensor.transpose(
                    ps_tr[:, m * P:(m + 1) * P],
                    xts[i][:, m * P:(m + 1) * P],
                    identity[:],
                )
            for m in range(NCT):
                eng = nc.vector if m % 2 == 0 else nc.scalar
                if m % 2 == 0:
                    eng.tensor_copy(
                        xT[m][:, i * P:(i + 1) * P], ps_tr[:, m * P:(m + 1) * P]
                    )
                else:
                    eng.copy(
                        xT[m][:, i * P:(i + 1) * P],
                        ps_tr[:, m * P:(m + 1) * P],
                    )

        # v = x @ Wv; vaug[i]: [128(t), 8, 65], [:, h, 64] = 1.0
        for i in range(NT):
            ps_v = ps1.tile([P, C], F32, tag="v", name="ps_v")
            for m in range(NCT):
                nc.tensor.matmul(
                    ps_v[:],
                    xT[m][:, i * P:(i + 1) * P],
                    wq[m][:, 2 * C:3 * C],
                    start=(m == 0),
                    stop=(m == NCT - 1),
                )
            va3 = vaug[i][:, 0:HEADS * VW].rearrange("p (h d) -> p h d", d=VW)
            nc.vector.tensor_copy(
                va3[:, :, 0:HC],
                ps_v[:].rearrange("p (h d) -> p h d", h=HEADS),
            )
            nc.vector.tensor_copy(va3[:, :, HC:VW], ones8[:])

        def emit_qk_mtile(m, psum_pool):
            """qkv-projection m-tile (q: m<4 -> qkT[m]; k: m>=4 -> kTz pair)."""
            ps_qk = psum_pool.tile([P, 2 * CHUNK], F32, tag="st", name="ps_bg")
            for j in range(NCH):
                for cc in range(NCT):
                    nc.tensor.matmul(
                        ps_qk[:, j * CHUNK:(j + 1) * CHUNK],
                        wq[cc][:, m * P:(m + 1) * P],
                        xT[cc][:, j * CHUNK:(j + 1) * CHUNK],
                        start=(cc == 0),
                        stop=(cc == NCT - 1),
                    )
            if m < NCT:
                nc.vector.tensor_scalar_add(qkT[m][:], ps_qk[:], qkb_t[m][:])
            else:
                hh = 2 * (m - NCT)
                nc.vector.tensor_scalar_add(
                    kTz[hh][0:HC, :], ps_qk[0:HC, :], qkb_t[m][0:HC]
                )
                nc.vector.tensor_scalar_add(
                    kTz[hh + 1][HC:P, :], ps_qk[HC:P, :], qkb_t[m][HC:P]
                )

        emit_qk_mtile(0, ps1)   # q for heads 0,1
        emit_qk_mtile(4, ps1)   # k for heads 0,1
        ps1_cm.__exit__(None, None, None)
        xin_cm.__exit__(None, None, None)

        # ================= phase 2: attention =================
        # Per head: 8 slots; slot g fills S^T for s-tile g (both 512-chunks)
        # into a 2-bank PSUM tile, ScalarE exps it into exh, and the PV
        # matmuls for slot g-1 (same head) ride behind on the PE.  One
        # background qkv m-tile per head is woven into the S^T PSUM ring.
        BG_ITEMS = [1, 5, 2, 6, 3, 7, None, None]  # per-head background m-tile

        with (
            tc.tile_pool(name="expsp", bufs=4) as expsp,
            tc.tile_pool(name="ps_st", bufs=2, space="PSUM") as ps_st,
            tc.tile_pool(name="ps_pv", bufs=2, space="PSUM") as ps_pv,
        ):
            def emit_pv(h, exs, pv_ps, g):
                """PV matmuls for s-tile g of head h (exs = that slot's exp)."""
                for j in range(NCH):
                    nc.tensor.matmul(
                        pv_ps[j][:],
                        vaug[g][:, h * VW:h * VW + P],
                        exs[:, j * CHUNK:(j + 1) * CHUNK],
                        start=(g == 0),
                        stop=(g == NT - 1),
                    )

            def emit_normalize(h, pv_ps):
                aoff = (h % 2) * HC
                am = h // 2
                dtmp = workp.tile([1, T], F32, tag="dtmp", name="dtmp")
                recip = workp.tile([1, T], F32, tag="recip", name="recip")
                bcast = workp.tile([HC, T], F32, tag="bcast", name="bcast")
                for j in range(NCH):
                    js = slice(j * CHUNK, (j + 1) * CHUNK)
                    nc.vector.tensor_copy(dtmp[:, js], pv_ps[j][HC:HC + 1, :])
                    nc.vector.reciprocal_approx_fast(recip[:, js], dtmp[:, js])
                    nc.gpsimd.partition_broadcast(
                        bcast[:, js], recip[:, js], channels=HC
                    )
                    nc.vector.tensor_tensor(
                        anorm[am][aoff:aoff + HC, js],
                        pv_ps[j][0:HC, :],
                        bcast[:, js],
                        op=mybir.AluOpType.mult,
                    )

            for h in range(HEADS):
                qm = h // 2
                pv_ps = [ps_pv.tile([P, CHUNK], F32, tag=f"pv{j}",
                                    name=f"pv{j}") for j in range(NCH)]
                exslots = []
                for g in range(NT):
                    st_ps = ps_st.tile([P, 2 * CHUNK], F32, tag="st", name="st")
                    for j in range(NCH):
                        nc.tensor.matmul(
                            st_ps[:, j * CHUNK:(j + 1) * CHUNK],
                            kTz[h][:, g * P:(g + 1) * P],
                            qkT[qm][:, j * CHUNK:(j + 1) * CHUNK],
                            start=True,
                            stop=True,
                        )
                    exs = expsp.tile([P, 2 * CHUNK], F32R, tag="exh",
                                     name="exh")
                    exslots.append(exs)
                    nc.scalar.activation(
                        exs[:],
                        st_ps[:],
                        mybir.ActivationFunctionType.Exp,
                        scale=EXP_SCALE,
                    )
                    if g >= 1:
                        emit_pv(h, exslots[g - 1], pv_ps, g - 1)
                    if g == 3 and BG_ITEMS[h] is not None:
                        emit_qk_mtile(BG_ITEMS[h], ps_st)
                emit_pv(h, exslots[NT - 1], pv_ps, NT - 1)
                emit_normalize(h, pv_ps)

        # ================= phase 3: out projection [t, e] =================
        with (
            tc.tile_pool(name="otp", bufs=2) as otp,
            tc.tile_pool(name="ps3", bufs=2, space="PSUM") as ps3,
        ):
            for i in range(NT):
                ps_o = ps3.tile([P, C], F32, tag="o", name="ps_o")
                for cc in range(NCT):
                    nc.tensor.matmul(
                        ps_o[:],
                        anorm[cc][:, i * P:(i + 1) * P],
                        wo[cc][:],
                        start=(cc == 0),
                        stop=(cc == NCT - 1),
                    )
                ot = otp.tile([P, C], F32, tag="ot", name="ot")
                nc.vector.tensor_tensor(
                    ot[:], ps_o[:], obb[:], op=mybir.AluOpType.add
                )
                nc.sync.dma_start(out_d.ap()[i * P:(i + 1) * P, :], ot[:])

    nc.compile()
    return nc


_CACHED_NC = None


def _get_nc():
    global _CACHED_NC
    if _CACHED_NC is None:
        _CACHED_NC = build_program()
    return _CACHED_NC


def kernel(x, qkv_w, qkv_b, out_w, out_b):
    """Full inputs in, full output out.  Shards batch across 8 NeuronCores."""
    from concourse.bass_utils import run_bass_kernel_spmd

    x = np.asarray(x)
    B, H, W, Cc = x.shape
    assert (B, H, W, Cc) == (8, 32, 32, C)
    x2 = np.ascontiguousarray(x.reshape(B, T, C).astype(np.float32))
    wq2 = np.asarray(qkv_w).reshape(C, 3 * C).astype(np.float32)
    wo2 = np.asarray(out_w).reshape(C, C).astype(np.float32)
    qkv_b = np.asarray(qkv_b).astype(np.float32)
    out_b = np.asarray(out_b).astype(np.float32)

    # host-side prep: tf32-round the weights (device loads them as float32r),
    # fold the v-bias through the output projection (exact: A_norm += b_v
    # shifts out by b_v @ W_out).
    wq_r = tf32_round(wq2)
    wo_r = tf32_round(wo2)
    b_v = qkv_b[2 * C:3 * C]
    ob_eff = (
        out_b.astype(np.float64) + b_v.astype(np.float64) @ wo_r.astype(np.float64)
    ).astype(np.float32)
    qkb = np.ascontiguousarray(qkv_b[0:2 * C])

    nc = _get_nc()
    in_maps = [
        {
            "x": np.ascontiguousarray(x2[b]),
            "qkv_w": np.ascontiguousarray(wq_r),
            "out_w": np.ascontiguousarray(wo_r),
            "qk_b": qkb,
            "out_b": ob_eff,
        }
        for b in range(B)
    ]
    trace = bool(int(os.environ.get("KERNEL_TRACE", "0")))
    res = run_bass_kernel_spmd(nc, in_maps, core_ids=list(range(B)), trace=trace)
    if trace and res.exec_time_ns is not None:
        print(f"HW exec time: {res.exec_time_ns} ns")
    kernel.last_results = res
    out = np.stack([res.results[b]["out"] for b in range(B)], axis=0)
    return out.reshape(B, H, W, Cc)


kernel.last_results = None
